# revision 51
# baseline (speedup 1.0000x reference)
"""Multi-head attention (B=2, N=2048, D=1024, H=16) on 8 trn2 NeuronCores.

Sharding: tensor-parallel over heads — core c owns heads (2c, 2c+1) for both
batches.  W_qkv columns / W_out rows are sliced per core on the host; each
core computes a full-size partial output (its heads' contribution through
W_out), and the host sums the 8 partials ("unshard" of the row-sharded W_out
product) and transposes back.

Device-side dataflow per core (all matmuls bf16, PSUM fp32):
  phase 1: QT = Wq^T x^T, KT = Wk^T x^T   ([dk*2, R] with dk on partitions)
           V  = x Wv                       ([R, dk*2] natural, + ones column)
  phase 2: scoresT_h = (KT_h)^T-matmul → [Nk, Nq] tiles; one exp per k-tile
           covering both heads (ACT, scale=1/sqrt(dk)); ctxT_h & softmax
           denominators from one M=65 matmul chain (lhsT = [V_h | m]); the
           0/1 attention mask is folded multiplicatively into V rows and the
           denominator column (exact); normalize via DVE reciprocal +
           gpsimd partition_broadcast.
  phase 3: outT_partial = Wo_c^T ctxT (+ b_out on core 0 only).
"""

import os
import sys
import functools

import numpy as np
import ml_dtypes

for _p in (
    "/root/.axon_site",
    "/root/.axon_site/_ro/trn_rl_repo",
    "/root/.axon_site/_ro/pypackages",
    "/opt/trn_rl_repo",
):
    if os.path.isdir(_p) and _p not in sys.path:
        sys.path.append(_p)

def _ensure_axon():
    """Register the axon PJRT backend if sitecustomize didn't run
    (e.g. kernel.py imported without the image's PYTHONPATH)."""
    import jax
    try:
        backends = jax._src.xla_bridge._backend_factories
        if "axon" in backends:
            return
    except Exception:
        pass
    try:
        from trn_agent_boot.trn_boot import boot
        os.environ.setdefault("AXON_POOL_SVC_OVERRIDE", "127.0.0.1")
        os.environ.setdefault("AXON_LOOPBACK_RELAY", "1")
        boot(os.environ["TRN_TERMINAL_PRECOMPUTED_JSON"],
             "/opt/axon/libaxon_pjrt.so")
    except Exception:
        pass


_ensure_axon()

import concourse.tile as tile
from concourse import bacc, mybir
from concourse.bass_utils import run_bass_kernel_spmd

BF16 = mybir.dt.bfloat16
F32 = mybir.dt.float32
I32 = mybir.dt.int32

B, N, D, H = 2, 2048, 1024, 16
DK = D // H            # 64
CORES = 8
HPC = H // CORES       # 2 heads per core
R = B * N              # 4096 rows total
RB = N                 # rows per batch
KT8 = D // 128         # 8 k-tiles over the model dim
CH = R // 512          # 8 row chunks of 512
QC = RB // 512         # 4 query chunks of 512 per batch
NKT = RB // 128        # 16 key tiles of 128 per batch
OT = D // 128          # 8 output-dim tiles


def _build_nc():
    nc = bacc.Bacc(
        "TRN2", target_bir_lowering=False, debug=False, enable_asserts=False,
        num_devices=CORES,
    )

    xT_d = nc.dram_tensor("xT", [D, R], BF16, kind="ExternalInput").ap()
    wq_d = nc.dram_tensor("wq", [128, KT8, 128], BF16, kind="ExternalInput").ap()
    wk_d = nc.dram_tensor("wk", [128, KT8, 128], BF16, kind="ExternalInput").ap()
    wv_d = nc.dram_tensor("wv", [128, KT8, 128], BF16, kind="ExternalInput").ap()
    wo_d = nc.dram_tensor("wo", [128, D], BF16, kind="ExternalInput").ap()
    bqkv_d = nc.dram_tensor("bqkv", [3, 128], F32, kind="ExternalInput").ap()
    maskT_d = nc.dram_tensor("maskT", [128, B * NKT], I32, kind="ExternalInput").ap()
    outT_d = nc.dram_tensor("outT", [D, R], F32, kind="ExternalOutput").ap()

    with tile.TileContext(nc) as tc:
        with (
            tc.tile_pool(name="persist", bufs=1) as persist,
            tc.tile_pool(name="xt", bufs=3) as xt_pool,
            tc.tile_pool(name="attn", bufs=5) as attn_pool,
            tc.tile_pool(name="small", bufs=4) as small,
            tc.tile_pool(name="outsb", bufs=2) as outsb_pool,
            tc.tile_pool(name="ps", bufs=2, space="PSUM") as ps_pool,
        ):
            # ---- constants / weights to SBUF ----
            wq_sb = persist.tile([128, KT8, 128], BF16, tag="wq")
            wk_sb = persist.tile([128, KT8, 128], BF16, tag="wk")
            wv_sb = persist.tile([128, KT8, 128], BF16, tag="wv")
            for w_sb, w_d in ((wq_sb, wq_d), (wk_sb, wk_d), (wv_sb, wv_d)):
                nc.sync.dma_start(out=w_sb, in_=w_d)
            wo_sb = persist.tile([128, D], BF16, tag="wo")

            bq_sb = persist.tile([128, 1], F32, tag="bq")
            bk_sb = persist.tile([128, 1], F32, tag="bk")
            bv_rep = persist.tile([128, 128], F32, tag="bv")
            maski = persist.tile([128, B * NKT], I32, tag="maski")
            maskf = persist.tile([128, B * NKT], F32, tag="maskf")

            def load_small_consts():
                nc.sync.dma_start(
                    out=bq_sb, in_=bqkv_d[0:1, :].rearrange("o p -> p o"))
                nc.sync.dma_start(
                    out=bk_sb, in_=bqkv_d[1:2, :].rearrange("o p -> p o"))
                nc.sync.dma_start(
                    out=bv_rep, in_=bqkv_d[2:3, :].partition_broadcast(128))
                nc.sync.dma_start(out=maski, in_=maskT_d)
                nc.vector.tensor_copy(out=maskf, in_=maski)


            # ---- persistent activations (split per batch for scheduling) ----
            qt_sb = [persist.tile([128, RB], BF16, tag=f"qt{b}", name=f"qt{b}") for b in range(B)]
            kt_sb = [persist.tile([128, RB], BF16, tag=f"kt{b}", name=f"kt{b}") for b in range(B)]
            # V rows with a ones column appended per head: [.., 64] = V, [64] = 1
            v_sb = [persist.tile([128, NKT, HPC, 66], BF16, tag=f"v{b}", name=f"v{b}")
                    for b in range(B)]

            ctxn_sb = [persist.tile([128, RB], BF16, tag=f"ctxn{b}", name=f"ctxn{b}")
                       for b in range(B)]

            # ---- phase 1: projections ----
            pending_v = {}

            def emit_v(b, chb, xt):
                    for sub in range(4):
                        rt = chb * 4 + sub
                        vps = ps_pool.tile([128, 128], F32, tag="mm1", name="vps")
                        for kt in range(KT8):
                            nc.tensor.matmul(
                                vps,
                                lhsT=xt[:, kt, sub * 128:(sub + 1) * 128],
                                rhs=wv_sb[:, kt, :],
                                start=(kt == 0), stop=(kt == KT8 - 1),
                            )
                        mcol = maskf[:, b * NKT + rt:b * NKT + rt + 1]
                        for h in range(HPC):
                            vslice = v_sb[b][:, rt, h, 0:64]
                            nc.vector.tensor_add(
                                out=vslice,
                                in0=vps[:, h * 64:(h + 1) * 64],
                                in1=bv_rep[:, h * 64:(h + 1) * 64],
                            )
                            nc.vector.tensor_scalar_mul(
                                out=vslice, in0=vslice, scalar1=mcol,
                            )
                            nc.vector.tensor_copy(
                                out=v_sb[b][:, rt, h, 64:65], in_=mcol,
                            )

            def phase1(b, chunks=None, after_dma=None):
                for chb in (chunks if chunks is not None else range(CH // B)):
                    roff = b * RB + chb * 512
                    xt = xt_pool.tile([128, KT8, 512], BF16, tag="xt")
                    for kt in range(KT8):
                        nc.sync.dma_start(
                            out=xt[:, kt, :],
                            in_=xT_d[kt * 128:(kt + 1) * 128, roff:roff + 512],
                        )
                    if after_dma is not None:
                        after_dma()
                        after_dma = None
                    for w_sb, b_sb, dest in (
                        (wq_sb, bq_sb, qt_sb[b]),
                        (wk_sb, bk_sb, kt_sb[b]),
                    ):
                        ps = ps_pool.tile([128, 512], F32, tag="mm1", name="qkvps")
                        for kt in range(KT8):
                            nc.tensor.matmul(
                                ps, lhsT=w_sb[:, kt, :], rhs=xt[:, kt, :],
                                start=(kt == 0), stop=(kt == KT8 - 1),
                            )
                        nc.vector.tensor_scalar_add(
                            out=dest[:, chb * 512:(chb + 1) * 512],
                            in0=ps, scalar1=b_sb,
                        )
                    if b in pending_v:
                        emit_v(b, *pending_v.pop(b))
                    pending_v[b] = (chb, xt)

            def phase1_flush(b):
                if b in pending_v:
                    emit_v(b, *pending_v.pop(b))

            # ---- phase 2: attention for batch b ----
            def attn_alloc():
                aw = [attn_pool.tile([128, NKT // 2, HPC, 512], BF16, tag="aw",
                                     name="awh") for _ in range(2)]
                cps = [ps_pool.tile([65, 512], F32, tag="ctxps", name=f"ctxps{h}")
                       for h in range(HPC)]
                return aw, cps

            def attn_scores(b, qc, aw, kt_lo, kt_hi, cps=None, interleave=False):
                qs = qc * 512
                for kt in range(kt_lo, kt_hi):
                    ks = kt * 128
                    sc = ps_pool.tile([128, HPC, 512], F32, tag="scps",
                                      name="scps")
                    for h in range(HPC):
                        nc.tensor.matmul(
                            sc[:, h, :],
                            lhsT=kt_sb[b][h * 64:(h + 1) * 64, ks:ks + 128],
                            rhs=qt_sb[b][h * 64:(h + 1) * 64, qs:qs + 512],
                            start=True, stop=True,
                            tile_position=(h * 64, 0),
                        )
                    nc.scalar.activation(
                        out=aw[kt // (NKT // 2)][:, kt % (NKT // 2), :, :],
                        in_=sc,
                        func=mybir.ActivationFunctionType.Exp,
                        scale=DK ** -0.5,
                    )
                    if interleave:
                        for h in range(HPC):
                            nc.tensor.matmul(
                                cps[h],
                                lhsT=v_sb[b][:, kt, h, 0:65],
                                rhs=aw[kt // (NKT // 2)][:, kt % (NKT // 2), h, :],
                                start=(kt == 0), stop=(kt == NKT - 1),
                                skip_group_check=True,
                            )

            def attn_ctx(b, aw, cps):
                for kt in range(NKT):
                    for h in range(HPC):
                        nc.tensor.matmul(
                            cps[h],
                            lhsT=v_sb[b][:, kt, h, 0:65],
                            rhs=aw[kt // (NKT // 2)][:, kt % (NKT // 2), h, :],
                            start=(kt == 0), stop=(kt == NKT - 1),
                        )

            def attn_norm(b, qc, cps):
                qs = qc * 512
                ctxfs = []
                for h in range(HPC):
                    ctxf = small.tile([65, 512], F32, tag="ctxf", name="ctxf")
                    nc.vector.tensor_copy(out=ctxf, in_=cps[h])
                    ctxfs.append(ctxf)
                for h in range(HPC):
                    ctxf = ctxfs[h]
                    rcp = small.tile([1, 512], F32, tag="rcp", name="rcp")
                    nc.vector.reciprocal(rcp, ctxf[64:65, :])
                    rep = small.tile([64, 512], F32, tag="rep", name="rep")
                    nc.gpsimd.partition_broadcast(rep, rcp)
                    nc.vector.tensor_mul(
                        out=ctxn_sb[b][h * 64:(h + 1) * 64, qs:qs + 512],
                        in0=ctxf[0:64, :], in1=rep,
                    )

            def phase2(b, qcs, after_chunk=None, mid_hooks=None):
                for qc in qcs:
                    last_chunk = (b == 1 and qc == QC - 1)
                    aw, cps = attn_alloc()
                    lo = 0
                    if last_chunk and mid_hooks:
                        for at_kt in sorted(mid_hooks):
                            attn_scores(b, qc, aw, lo, at_kt, cps=cps,
                                        interleave=True)
                            mid_hooks[at_kt]()
                            lo = at_kt
                    attn_scores(b, qc, aw, lo, NKT, cps=cps,
                                interleave=last_chunk)
                    if not last_chunk:
                        attn_ctx(b, aw, cps)
                    attn_norm(b, qc, cps)
                    if after_chunk is not None:
                        after_chunk(qc)

            # ---- phase 3: output projection (partial, this core's heads) ----
            def phase3(b, rc):
                osb = outsb_pool.tile([128, OT, 512], F32, tag="osb")
                tail = (b == 1 and rc == QC - 1)
                for ot in range(OT):
                    ps = ps_pool.tile([128, 512], F32, tag="mm1", name="outps")
                    nc.tensor.matmul(
                        ps,
                        lhsT=wo_sb[:, ot * 128:(ot + 1) * 128],
                        rhs=ctxn_sb[b][:, rc * 512:(rc + 1) * 512],
                        start=True, stop=True,
                    )
                    if tail and ot % 2 == 1:
                        nc.scalar.copy(out=osb[:, ot, :], in_=ps)
                    else:
                        nc.vector.tensor_copy(out=osb[:, ot, :], in_=ps)
                cs = b * RB + rc * 512
                outT_r = outT_d.rearrange("(t p) r -> p t r", p=128)
                step = 1 if tail else 2
                for j in range(0, OT, step):
                    nc.sync.dma_start(
                        out=outT_r[:, j:j + step, cs:cs + 512],
                        in_=osb[:, j:j + step, :],
                    )

            load_small_consts()
            phase1(0, chunks=[0])
            aw00, cps00 = attn_alloc()
            attn_scores(0, 0, aw00, 0, 4)
            phase1(0, chunks=[1])
            attn_scores(0, 0, aw00, 4, 8)
            phase1(0, chunks=[2])
            attn_scores(0, 0, aw00, 8, 12)
            phase1(0, chunks=[3])
            attn_scores(0, 0, aw00, 12, NKT)
            phase1_flush(0)
            attn_ctx(0, aw00, cps00)
            attn_norm(0, 0, cps00)
            phase2(0, range(1, QC))
            nc.sync.dma_start(out=wo_sb, in_=wo_d)
            phase1(1, chunks=[0])
            aw10, cps10 = attn_alloc()
            attn_scores(1, 0, aw10, 0, 4)
            phase1(1, chunks=[1])
            attn_scores(1, 0, aw10, 4, 8)
            phase1(1, chunks=[2])
            attn_scores(1, 0, aw10, 8, 12)
            phase1(1, chunks=[3])
            attn_scores(1, 0, aw10, 12, NKT)
            phase1_flush(1)
            attn_ctx(1, aw10, cps10)
            attn_norm(1, 0, cps10)
            phase3(0, 0)
            phase3(0, 1)
            phase3(1, 0)

            def tail_chunks(qc):
                if qc == 1:
                    phase3(0, 2)
                if qc != 2:
                    phase3(1, qc)

            phase2(1, range(1, QC), after_chunk=tail_chunks,
                   mid_hooks={8: lambda: phase3(1, 2),
                              12: lambda: phase3(0, 3)})

    nc.compile()
    return nc


@functools.lru_cache(maxsize=1)
def _get_nc():
    return _build_nc()


def _make_in_maps(x, attention_mask, W_qkv, b_qkv, W_out, b_out):
    bf16 = ml_dtypes.bfloat16
    x = np.asarray(x, dtype=np.float32).reshape(R, D)
    xT = np.ascontiguousarray(x.T).astype(bf16)
    W_qkv = np.asarray(W_qkv, dtype=np.float32)
    W_out = np.asarray(W_out, dtype=np.float32)
    b_qkv = np.asarray(b_qkv, dtype=np.float32)
    b_out = np.asarray(b_out, dtype=np.float32)
    mask = np.asarray(attention_mask).astype(np.int32)
    maskT = np.ascontiguousarray(
        mask.reshape(B, NKT, 128).transpose(2, 0, 1).reshape(128, B * NKT)
    )

    def _ktile(w):  # [1024, 128] -> [128(p), 8(t), 128(m)]
        return np.ascontiguousarray(
            w.reshape(KT8, 128, 128).transpose(1, 0, 2)).astype(bf16)

    in_maps = []
    for c in range(CORES):
        s = slice(128 * c, 128 * (c + 1))
        in_maps.append({
            "xT": xT,
            "wq": _ktile(W_qkv[:, s]),
            "wk": _ktile(W_qkv[:, D:][:, s]),
            "wv": _ktile(W_qkv[:, 2 * D:][:, s]),
            "wo": np.ascontiguousarray(W_out[s, :]).astype(bf16),
            "bqkv": np.ascontiguousarray(
                np.stack([b_qkv[s], b_qkv[D:][s], b_qkv[2 * D:][s]])
            ),
            "maskT": maskT,
        })
    return in_maps


def timeline_estimate_ns():
    """Cost-model makespan of the per-core program (no HW needed)."""
    from concourse.timeline_sim import TimelineSim
    return TimelineSim(_get_nc(), trace=False).simulate()


def run(trace=False, **inputs):
    nc = _get_nc()
    b_out = np.asarray(inputs["b_out"], dtype=np.float32)
    in_maps = _make_in_maps(**inputs)
    try:
        res = run_bass_kernel_spmd(
            nc, in_maps, core_ids=list(range(CORES)), trace=trace,
        )
    except (ImportError, ModuleNotFoundError):
        # NTFF profiling hook unavailable in this client image
        res = run_bass_kernel_spmd(
            nc, in_maps, core_ids=list(range(CORES)), trace=False,
        )
    acc = np.zeros((D, R), dtype=np.float32)
    for r in res.results:
        acc += r["outT"]
    out = (np.ascontiguousarray(acc.T) + b_out).reshape(B, N, D)
    return out, res


def kernel(**inputs):
    out, _ = run(trace=False, **inputs)
    return out
